# revision 5
# baseline (speedup 1.0000x reference)
"""Bidirectional multi-head attention on 8 Trainium2 NeuronCores.

Problem: x:(2,2048,1024) f32, 16 heads of 64; qkv proj -> attention with
key-padding mask -> softmax -> out proj.  Sharding: batch (2) x head-groups
(4 groups of 4 heads) = 8 cores.  Each core computes its 4 heads' attention
context and a partial output projection (over its 256 context channels);
the host sums the 4 partial projections per batch (pure unshard + add).

On-chip layout is fully "transposed" (features on partitions, sequence on
the free axis), which makes every matmul contraction land on partitions
without any on-chip transposes:
  Q^T,K^T = W x^T          (lhsT = W^T tiles, rhs = x^T)
  V       = x W^T          (lhsT = x^T tiles, rhs = Wv^T)   [normal orient]
  S^T     = K^T' Q^T       (per 128-key tile; two heads row-tiled per pass)
  P^T     = exp(S^T/8 + maskbias[k])   [mask folded into per-partition bias]
  O_aug^T = V_aug^T P^T    (V_aug = [V | 1]; row 64 = softmax denominator)
  out^T  += Wp^T ctx^T     (partial over this core's 256 channels)

Softmax skips the running-max (scores are bounded: |s/8| < 4 for this
problem's scale) and folds the key mask into the exp bias (-30 => exp~0).
The denominator arrives for free as V_aug's ones-column, and 1/den is
partition-broadcast via a tiny SBUF->SBUF DMA.
"""

import numpy as np

import bass_rust
import concourse.bass as bass
import concourse.mybir as mybir
import concourse.tile as tile
from concourse.bass_utils import run_bass_kernel_spmd
from concourse.vector_clock import ScopedClock

F32 = mybir.dt.float32
AF = mybir.ActivationFunctionType

B, L, D, H, HD = 2, 2048, 1024, 16, 64
GROUPS = 4            # head groups per batch (one per core)
HPG = H // GROUPS     # 4 heads per group
CH = HPG * HD         # 256 context channels per group
NQB = L // 512        # q blocks of 512
NKT = L // 128        # k tiles of 128
NC_ = D // 128        # contraction chunks of 128 over the model dim
SCALE = 1.0 / float(np.sqrt(HD))

MAXW = 1  # this walrus build accepts only ONE embedded sync wait per inst


class PatchedTileContext(tile.TileContext):
    """TileContext for walrus builds limited to one sync wait per
    instruction: excess waits move onto same-engine carrier NoOps committed
    immediately before the owning instruction (engines execute in order, so
    the wait set is honored at the same program point)."""

    def _split_waits(self, inst):
        si = inst.sync_info
        if si is None:
            return None
        waits = list(si.on_wait)
        if len(waits) <= MAXW:
            return None
        inst.sync_info = bass_rust.SyncInfo(
            on_wait=waits[-MAXW:], on_update=list(si.on_update)
        )
        carriers = []
        for i in range(0, len(waits) - MAXW, MAXW):
            nop = mybir.InstNoOp(
                name=self.nc.get_next_instruction_name(),
                engine=inst.engine,
                bass_nofuse=True,
            )
            nop.sync_info = bass_rust.SyncInfo(on_wait=waits[i : i + MAXW], on_update=[])
            carriers.append(nop)
        return carriers

    def _commit_instruction(self, inst, lazy_reg_writes: bool = True):
        carriers = self._split_waits(inst)
        if carriers:
            for nop in carriers:
                super()._commit_instruction(nop)
        return super()._commit_instruction(inst, lazy_reg_writes)

    def _drain_and_barrier(self, tick_clock, wait_clock):
        drain_inst = self.nc.sync.drain()
        wait_clock.add_sem_waits(
            drain_inst.ins, ScopedClock({None: tick_clock.global_clock})
        )
        waits = list(drain_inst.ins.sync_info.on_wait)
        if len(waits) > MAXW:
            drain_inst.ins.sync_info = bass_rust.SyncInfo(
                on_wait=waits[:MAXW], on_update=[]
            )
            for i in range(MAXW, len(waits), MAXW):
                extra = self.nc.sync.drain()
                extra.ins.sync_info = bass_rust.SyncInfo(
                    on_wait=waits[i : i + MAXW], on_update=[]
                )
        self.nc.all_engine_barrier()
        assert self.sems is not None
        popped = self.nc._tile_sem_poison_stack.pop()
        assert popped is self._sem_poison
        self.nc.clear_and_free_semaphores(list(self.sems.allocated().values()))
        self.nc.all_engine_barrier()


def _build_nc():
    nc = bass.Bass()
    xT_h = nc.dram_tensor("xT", [D, L], F32, kind="ExternalInput")
    wqkT_h = nc.dram_tensor("wqkT", [D, 2 * CH], F32, kind="ExternalInput")
    wvT_h = nc.dram_tensor("wvT", [D, CH], F32, kind="ExternalInput")
    wpT_h = nc.dram_tensor("wpT", [CH, D], F32, kind="ExternalInput")
    bqk_h = nc.dram_tensor("bqk", [128, 4], F32, kind="ExternalInput")
    bvb_h = nc.dram_tensor("bvb", [128, CH], F32, kind="ExternalInput")
    bp_h = nc.dram_tensor("bp", [128, 8], F32, kind="ExternalInput")
    mb_h = nc.dram_tensor("mb", [128, NKT], F32, kind="ExternalInput")
    outT_h = nc.dram_tensor("outT", [D, L], F32, kind="ExternalOutput")

    with PatchedTileContext(nc) as tc:
        with (
            tc.tile_pool(name="consts", bufs=1) as consts,
            tc.tile_pool(name="persist", bufs=1) as persist,
        ):
            # small constants
            bqk_sb = consts.tile([128, 4], F32)
            nc.sync.dma_start(bqk_sb[:], bqk_h[:])
            bvb_sb = consts.tile([128, HPG, HD], F32)
            nc.sync.dma_start(bvb_sb[:], bvb_h[:].rearrange("p (h d) -> p h d", h=HPG))
            bp_sb = consts.tile([128, 8], F32)
            nc.sync.dma_start(bp_sb[:], bp_h[:])
            mb_sb = consts.tile([128, NKT], F32)
            nc.sync.dma_start(mb_sb[:], mb_h[:])
            wp_sb = consts.tile([128, 2, D], F32)
            nc.sync.dma_start(wp_sb[:], wpT_h[:].rearrange("(c p) m -> p c m", p=128))

            # persistent activations
            QT_sb = persist.tile([128, 2, L], F32)   # [64*head-pair-lane, hp, q]
            KT_sb = persist.tile([128, 2, L], F32)
            Vaug_sb = persist.tile([128, NKT, HPG, HD + 1], F32)
            ctxT_sb = persist.tile([128, 2, L], F32)
            nc.vector.memset(Vaug_sb[:, :, :, HD : HD + 1], 1.0)

            # ---------------- phase A: QKV projections ----------------
            with (
                tc.tile_pool(name="xw", bufs=1) as xw,
                tc.tile_pool(name="qkv_ps", bufs=3, space="PSUM") as qkv_ps,
            ):
                wqk_sb = xw.tile([128, NC_, 2 * CH], F32)
                nc.sync.dma_start(
                    wqk_sb[:], wqkT_h[:].rearrange("(c p) m -> p c m", p=128)
                )
                wv_sb = xw.tile([128, NC_, CH], F32)
                nc.sync.dma_start(
                    wv_sb[:], wvT_h[:].rearrange("(c p) m -> p c m", p=128)
                )
                xT_sb = xw.tile([128, NC_, L], F32)
                xT_r = xT_h[:].rearrange("(c p) l -> p c l", p=128)
                for c in range(NC_):
                    nc.sync.dma_start(xT_sb[:, c, :], xT_r[:, c, :])

                # Q^T and K^T: 4 m-tiles (q-hp0, q-hp1, k-hp0, k-hp1)
                for mt in range(4):
                    for lb in range(NQB):
                        ps = qkv_ps.tile([128, 512], F32, tag="qk")
                        for c in range(NC_):
                            nc.tensor.matmul(
                                ps[:],
                                wqk_sb[:, c, mt * 128 : (mt + 1) * 128],
                                xT_sb[:, c, lb * 512 : (lb + 1) * 512],
                                start=(c == 0),
                                stop=(c == NC_ - 1),
                            )
                        dst = QT_sb if mt < 2 else KT_sb
                        nc.vector.tensor_scalar_add(
                            out=dst[:, mt % 2, lb * 512 : (lb + 1) * 512],
                            in0=ps[:],
                            scalar1=bqk_sb[:, mt : mt + 1],
                        )

                # V in normal orientation, into V_aug's first 64 columns
                for lt in range(NKT):
                    ps = qkv_ps.tile([128, CH], F32, tag="v")
                    for c in range(NC_):
                        nc.tensor.matmul(
                            ps[:],
                            xT_sb[:, c, lt * 128 : (lt + 1) * 128],
                            wv_sb[:, c, :],
                            start=(c == 0),
                            stop=(c == NC_ - 1),
                        )
                    nc.vector.tensor_add(
                        out=Vaug_sb[:, lt, :, 0:HD],
                        in0=ps[:].rearrange("p (h d) -> p h d", h=HPG),
                        in1=bvb_sb[:],
                    )

            # ---------------- phase B: attention ----------------
            with (
                tc.tile_pool(name="s_ps", bufs=2, space="PSUM") as s_ps_pool,
                tc.tile_pool(name="o_ps", bufs=2, space="PSUM") as o_ps_pool,
                tc.tile_pool(name="p_sb", bufs=3) as p_pool,
                tc.tile_pool(name="norm", bufs=2) as norm_pool,
                tc.tile_pool(name="norm_dr", bufs=2, space="DRAM") as norm_dr,
            ):
                for hp in range(2):
                    for qb in range(NQB):
                        qsl = slice(qb * 512, (qb + 1) * 512)
                        o_ps = o_ps_pool.tile([HD + 1, 2, 512], F32, tag="o")
                        for kt in range(NKT):
                            ksl = slice(kt * 128, (kt + 1) * 128)
                            s_ps = s_ps_pool.tile([128, 2, 512], F32, tag="s")
                            nc.tensor.matmul(
                                s_ps[:, 0, :],
                                KT_sb[0:64, hp, ksl],
                                QT_sb[0:64, hp, qsl],
                                start=True,
                                stop=True,
                            )
                            nc.tensor.matmul(
                                s_ps[:, 1, :],
                                KT_sb[64:128, hp, ksl],
                                QT_sb[64:128, hp, qsl],
                                start=True,
                                stop=True,
                                tile_position=(64, 0),
                            )
                            p_sb = p_pool.tile([128, 2, 512], F32, tag="p")
                            nc.scalar.activation(
                                out=p_sb[:],
                                in_=s_ps[:],
                                func=AF.Exp,
                                bias=mb_sb[:, kt : kt + 1],
                                scale=float(SCALE),
                            )
                            for hh in range(2):
                                nc.tensor.matmul(
                                    o_ps[:, hh, :],
                                    Vaug_sb[:, kt, 2 * hp + hh, :],
                                    p_sb[:, hh, :],
                                    start=(kt == 0),
                                    stop=(kt == NKT - 1),
                                    skip_group_check=True,
                                )
                        # normalize: ctx^T = O_aug[0:64] * (1/den)
                        r_sb = norm_pool.tile([1, 1, 2, 512], F32, tag="r")
                        nc.vector.reciprocal(
                            out=r_sb[:, 0, :, :], in_=o_ps[HD : HD + 1, :, :]
                        )
                        r_dr = norm_dr.tile([1, 2, 512], F32, tag="rd")
                        nc.sync.dma_start(r_dr[:], r_sb[:, 0, :, :])
                        bc_sb = norm_pool.tile([64, 1, 2, 512], F32, tag="bc")
                        nc.sync.dma_start(
                            bc_sb[:, 0, :, :], r_dr[:].to_broadcast((64, 2, 512))
                        )
                        for hh in range(2):
                            nc.vector.tensor_mul(
                                out=ctxT_sb[hh * 64 : (hh + 1) * 64, hp, qsl],
                                in0=o_ps[0:HD, hh, :],
                                in1=bc_sb[:, 0, hh, :],
                            )

            # ---------------- phase C: output projection (partial) ----------
            with (
                tc.tile_pool(name="pr_ps", bufs=3, space="PSUM") as pr_ps,
                tc.tile_pool(name="stage", bufs=3) as stage,
            ):
                for mt in range(8):
                    msl = slice(mt * 128, (mt + 1) * 128)
                    for qb in range(NQB):
                        qsl = slice(qb * 512, (qb + 1) * 512)
                        ps = pr_ps.tile([128, 512], F32, tag="pr")
                        for hp in range(2):
                            nc.tensor.matmul(
                                ps[:],
                                wp_sb[:, hp, msl],
                                ctxT_sb[:, hp, qsl],
                                start=(hp == 0),
                                stop=(hp == 1),
                            )
                        st = stage.tile([128, 512], F32, tag="st")
                        nc.scalar.activation(
                            out=st[:],
                            in_=ps[:],
                            func=AF.Identity,
                            bias=bp_sb[:, mt : mt + 1],
                            scale=1.0,
                        )
                        nc.sync.dma_start(outT_h[msl, qsl], st[:])
    return nc


_NC_CACHE = None


def _get_nc():
    global _NC_CACHE
    if _NC_CACHE is None:
        _NC_CACHE = _build_nc()
    return _NC_CACHE


def _prep_core_inputs(core, x, mask, wqkv, bqkv, wproj, bproj):
    b, g = core // GROUPS, core % GROUPS
    sl = slice(g * CH, (g + 1) * CH)
    wq = wqkv[0 * D + g * CH : 0 * D + (g + 1) * CH]
    wk = wqkv[1 * D + g * CH : 1 * D + (g + 1) * CH]
    wv = wqkv[2 * D + g * CH : 2 * D + (g + 1) * CH]
    bq = bqkv[0 * D + g * CH : 0 * D + (g + 1) * CH]
    bk = bqkv[1 * D + g * CH : 1 * D + (g + 1) * CH]
    bv = bqkv[2 * D + g * CH : 2 * D + (g + 1) * CH]
    bpc = bproj if g == 0 else np.zeros_like(bproj)
    mb = np.where(mask[b], np.float32(-30.0), np.float32(0.0))
    return {
        "xT": np.ascontiguousarray(x[b].T),
        "wqkT": np.ascontiguousarray(np.concatenate([wq, wk], axis=0).T),
        "wvT": np.ascontiguousarray(wv.T),
        "wpT": np.ascontiguousarray(wproj[:, sl].T),
        "bqk": np.ascontiguousarray(
            np.concatenate([bq, bk]).reshape(4, 128).T
        ),
        "bvb": np.ascontiguousarray(np.broadcast_to(bv, (128, CH))),
        "bp": np.ascontiguousarray(bpc.reshape(8, 128).T),
        "mb": np.ascontiguousarray(mb.reshape(NKT, 128).T),
    }


def kernel(x, mask, wqkv, bqkv, wproj, bproj, _trace=False, _trace_kwargs=None):
    x = np.asarray(x, np.float32)
    mask = np.asarray(mask, bool)
    wqkv = np.asarray(wqkv, np.float32)
    bqkv = np.asarray(bqkv, np.float32)
    wproj = np.asarray(wproj, np.float32)
    bproj = np.asarray(bproj, np.float32)

    nc = _get_nc()
    in_maps = [
        _prep_core_inputs(c, x, mask, wqkv, bqkv, wproj, bproj) for c in range(8)
    ]
    kw = {}
    if _trace:
        kw = {"trace": True, **(_trace_kwargs or {})}
    res = run_bass_kernel_spmd(nc, in_maps, list(range(8)), **kw)
    out = np.empty((B, L, D), np.float32)
    for b in range(B):
        acc = np.array(res.results[b * GROUPS + 0]["outT"], np.float32)
        for g in range(1, GROUPS):
            acc += res.results[b * GROUPS + g]["outT"]
        out[b] = acc.T
    if _trace:
        return out, res
    return out
